# revision 1
# baseline (speedup 1.0000x reference)
"""Causal single-head attention on 8 Trainium2 NeuronCores.

Problem: x[8, 2048, 1024] -> out[8, 2048, 64]
  q/k/v = x @ W{q,k,v} + b{q,k,v};  out = softmax(causal(q k^T / 8)) v

Sharding: data-parallel over batch; core b computes batch element b.

Per-core design (T=2048, D=1024, H=64), all matmul operands bf16 with
fp32 PSUM accumulation; transposes and the output path in f32r/f32:
  - host sends x[b]^T as xt [D, T] (bf16) so D sits on partitions
  - QKV: lhsT=[Wq|Wk] chunk [128d, 128] (full PE array), rhs=xt chunk
    [128d, 512t] -> psum [128(qh|kh), 512]; bias added during the
    PSUM->SBUF copy, giving qT/kT [64h, T].  V: lhsT=Wv (M=64), then
    PE-transposed to natural v tiles [128t, 64h] + ones/zeros columns
    -> v_sb [128, 16, 66].
  - attention in four i-quarters of 512: for each quarter iq and each
    causal j-chunk jt (128 rows): S^T psum [128j, <=512i] = kT_c.T@qT;
    exp via ACT (scale=1/8 fused) psum->P (bf16); diagonal 128-block
    causally masked by affine_select; PV accumulates
    psum_out[66, 512] += [v_jt|1|0].T @ P, whose row 64 is the softmax
    denominator for free.
  - out: PE-transpose [66,128] blocks -> [128, 66], divide rows by the
    denominator column (per-partition scalar after transpose), DMA out.
  - all PSUM tiles are one bank (512 f32) -> 6 rotating work slots + 2
    output-accumulator slots; PE warmup matmuls + an early dummy exp
    keep the HAM clock gate at 8/8 and preload the ACT table.
"""

import os
from contextlib import ExitStack

import ml_dtypes
import numpy as np

import concourse.bacc as bacc
import concourse.mybir as mybir
import concourse.tile as tile
from concourse.bass_utils import run_bass_kernel_spmd

F32 = mybir.dt.float32
F32R = mybir.dt.float32r
BF16 = mybir.dt.bfloat16
AF = mybir.ActivationFunctionType
ALU = mybir.AluOpType

T = 2048
D = 1024
H = 64
NB = 8
DC = D // 128      # 8 contraction chunks
NJT = T // 128     # 16 j-chunks (also 16 t-tiles)
QW = 512           # i-quarter width
NQ = T // QW       # 4 quarters
SCALE = 1.0 / 8.0  # 1/sqrt(H)

_CACHE: dict = {}


def _emit_v_group(nc, lo, xt_sb, wv_sb, bv_sb, vT, ps):
    sl = slice(lo, lo + 512)
    ps_v = ps.tile([64, 512], F32, tag="s", name=f"psv{lo}")
    for c in range(DC):
        nc.tensor.matmul(
            ps_v[:], wv_sb[:, c, :], xt_sb[:, c, sl],
            start=(c == 0), stop=(c == DC - 1),
        )
    nc.vector.tensor_scalar(
        out=vT[:, sl], in0=ps_v[:],
        scalar1=bv_sb[:], scalar2=None, op0=ALU.add,
    )


def _emit_v_transpose(nc, lo, vT, v_sb, ident, ps):
    # transpose v^T [64, 128]-tiles -> natural v [128, 64] tiles
    ps_t = ps.tile([128, 4, H], F32, tag="s", name=f"pst{lo}")
    for j2 in range(4):
        jt = lo // 128 + j2
        nc.tensor.transpose(
            ps_t[:, j2, :].bitcast(F32R),
            vT[:, jt * 128:(jt + 1) * 128],
            ident[0:64, 0:64],
        )
    nc.vector.tensor_copy(
        v_sb[:, lo // 128:lo // 128 + 4, 0:H], ps_t[:, :, :])


def _emit_qk_group(nc, lo, xt_sb, wqk_sb, bqk_sb, qT, kT, ps):
    sl = slice(lo, lo + 512)
    ps_qk = ps.tile([128, 512], F32, tag="s", name=f"psqk{lo}")
    for c in range(DC):
        nc.tensor.matmul(
            ps_qk[:], wqk_sb[:, c, :], xt_sb[:, c, sl],
            start=(c == 0), stop=(c == DC - 1),
        )
    nc.vector.tensor_scalar(
        out=qT[:, sl], in0=ps_qk[0:64, :],
        scalar1=bqk_sb[0:64, :], scalar2=None, op0=ALU.add,
    )
    nc.vector.tensor_scalar(
        out=kT[:, sl], in0=ps_qk[64:128, :],
        scalar1=bqk_sb[64:128, :], scalar2=None, op0=ALU.add,
    )


def _emit_qkv_half(nc, th, xt_sb, wqk_sb, wv_sb, bqk_sb, bv_sb,
                   qT, kT, vT, v_sb, ident, ps, defer_v=False,
                   defer_qk=False):
    """QKV for t in [th*1024, (th+1)*1024); optionally defer QK/V passes."""
    qk_closures = []
    v_closures = []
    for t2 in range(2):
        lo = th * 1024 + t2 * 512
        if defer_qk:
            qk_closures.append(
                lambda lo=lo: _emit_qk_group(nc, lo, xt_sb, wqk_sb, bqk_sb,
                                             qT, kT, ps))
        else:
            _emit_qk_group(nc, lo, xt_sb, wqk_sb, bqk_sb, qT, kT, ps)
        if defer_v:
            v_closures.append(
                lambda lo=lo: _emit_v_group(nc, lo, xt_sb, wv_sb, bv_sb,
                                            vT, ps))
            v_closures.append(
                lambda lo=lo: _emit_v_transpose(nc, lo, vT, v_sb, ident, ps))
        else:
            _emit_v_group(nc, lo, xt_sb, wv_sb, bv_sb, vT, ps)
            _emit_v_transpose(nc, lo, vT, v_sb, ident, ps)
    return qk_closures, v_closures


def _emit_attn_pair(nc, iqa, iqb, qT, kT, v_sb, ps, out_ps, ppool,
                    otpool=None, fillers=None):
    """One i-halfwindow [iqa*QW, (iqb+1)*QW) per 1024-wide strip: one exp
    per j-chunk (halves ACT fixed cost), PV split into the two quarter
    accumulators.

    `fillers`: optional list of zero-arg closures (deferred real work);
    one fires after each jt round to fill PE exp-wait slivers and keep
    the HAM clock at 8/8.
    """
    assert iqb == iqa + 1
    w0 = iqa * QW                       # window start in i
    jt_max = min(NJT, 4 * (iqb + 1))    # causal j-chunks for the window
    jma = min(NJT, 4 * (iqa + 1))       # last writer of quarter a is jma-1
    ps_oa = out_ps.tile([66, QW], F32, tag="out")
    ps_ob = out_ps.tile([66, QW], F32, tag="out")
    for jt in range(jt_max):
        off = max(128 * jt - w0, 0)     # within [0, 1024)
        ps_s = ps.tile([128, 2 * QW], F32, tag="w")
        for h in range(2):
            a, b = max(off, h * QW), (h + 1) * QW
            if a < b:
                nc.tensor.matmul(
                    ps_s[:, a:b],
                    kT[:, jt * 128:(jt + 1) * 128],
                    qT[:, w0 + a: w0 + b],
                    start=True, stop=True,
                )
        P = ppool.tile([128, 2 * QW], BF16, tag="P")
        nc.scalar.activation(
            out=P[:, off:], in_=ps_s[:, off:], func=AF.Exp, scale=SCALE,
        )
        if 128 * jt >= w0:
            # diagonal block: keep i >= j  (i = w0+off+f, j = 128*jt+p)
            nc.gpsimd.affine_select(
                out=P[:, off:off + 128], in_=P[:, off:off + 128],
                compare_op=ALU.is_ge, fill=0.0,
                base=0, pattern=[[1, 128]], channel_multiplier=-1,
            )
        if off < QW and jt < jma:
            nc.tensor.matmul(
                ps_oa[:, off:QW], v_sb[:, jt, :], P[:, off:QW],
                start=(jt == 0), stop=(jt == jma - 1),
            )
        offb = max(off - QW, 0)
        nc.tensor.matmul(
            ps_ob[:, offb:QW], v_sb[:, jt, :], P[:, QW + offb:],
            start=(jt == 0), stop=(jt == jt_max - 1),
        )
        if fillers:
            fillers.pop(0)()
    return ps_oa, ps_ob


def _emit_drain_copy(nc, otpool, ps_o):
    oT = otpool.tile([66, QW], F32R, tag="oT")
    nc.vector.tensor_copy(oT[:], ps_o[:])
    return oT


def _drain_closures(nc, iq, oT, ident, ps, out_nat, recip, out):
    """Drain work as closures: 4 transposes + 1 finish step."""
    state = {}

    def _tr(t2):
        def go():
            if "ps_n" not in state:
                state["ps_n"] = ps.tile([128, 4, 128], F32, tag="s", name=f"psn{iq}")
            nc.tensor.transpose(
                state["ps_n"][:, t2, 0:66].bitcast(F32R),
                oT[:, t2 * 128:(t2 + 1) * 128],
                ident[:, :],
            )
        return go

    def _fin():
        sl = slice(iq * 4, (iq + 1) * 4)
        nc.vector.tensor_copy(out_nat[:, sl, :], state["ps_n"][:, :, 0:66])
        nc.vector.reciprocal(recip[:, sl], out_nat[:, sl, H])
        for tt in range(iq * 4, (iq + 1) * 4):
            nc.vector.tensor_scalar_mul(
                out_nat[:, tt, 0:H], out_nat[:, tt, 0:H],
                recip[:, tt:tt + 1])
        nc.sync.dma_start(
            out=out.rearrange("(qq tt p) h -> qq p tt h", qq=NQ, p=128)[iq],
            in_=out_nat[:, sl, 0:H],
        )

    return [_tr(t) for t in range(4)] + [_fin]


def _emit_drain(nc, iq, oT, ident, ps, out_nat, recip, out):
    for go in _drain_closures(nc, iq, oT, ident, ps, out_nat, recip, out):
        go()


def _build():
    nc = bacc.Bacc("TRN2", target_bir_lowering=False, debug=False,
                   num_devices=NB)
    xt = nc.dram_tensor("xt", [D, T], BF16, kind="ExternalInput")
    wqk = nc.dram_tensor("wqk", [D, 128], BF16, kind="ExternalInput")
    wv = nc.dram_tensor("wv", [D, H], BF16, kind="ExternalInput")
    bqk = nc.dram_tensor("bqk", [128, 1], F32, kind="ExternalInput")
    bv = nc.dram_tensor("bv", [H, 1], F32, kind="ExternalInput")
    ident66 = nc.dram_tensor("ident66", [66, 66], F32R, kind="ExternalInput")
    vtail = nc.dram_tensor("vtail", [128, NJT, 2], BF16, kind="ExternalInput")
    out = nc.dram_tensor("out", [T, H], F32, kind="ExternalOutput")

    with ExitStack() as ctx:
        tc = ctx.enter_context(tile.TileContext(nc))
        const = ctx.enter_context(tc.tile_pool(name="const", bufs=1))
        big = ctx.enter_context(tc.tile_pool(name="big", bufs=1))
        ppool = ctx.enter_context(tc.tile_pool(name="ppool", bufs=4))
        otpool = ctx.enter_context(tc.tile_pool(name="otpool", bufs=2))
        ps = ctx.enter_context(tc.tile_pool(name="ps", bufs=2, space="PSUM"))
        pss = ctx.enter_context(tc.tile_pool(name="pss", bufs=2, space="PSUM"))
        out_ps = ctx.enter_context(
            tc.tile_pool(name="out_ps", bufs=2, space="PSUM"))

        # constants / weights
        wqk_sb = const.tile([128, DC, 128], BF16)
        nc.sync.dma_start(
            out=wqk_sb[:], in_=wqk.rearrange("(c p) m -> p c m", p=128))
        wv_sb = const.tile([128, DC, H], BF16)
        nc.sync.dma_start(
            out=wv_sb[:], in_=wv.rearrange("(c p) m -> p c m", p=128))
        bqk_sb = const.tile([128, 1], F32)
        nc.sync.dma_start(out=bqk_sb[:], in_=bqk[:])
        bv_sb = const.tile([H, 1], F32)
        nc.sync.dma_start(out=bv_sb[:], in_=bv[:])
        ident = const.tile([66, 66], F32R)
        nc.sync.dma_start(out=ident[:], in_=ident66[:])

        # x^T resident in SBUF (bf16), th-major halves for early compute
        xt_sb = big.tile([128, DC, T], BF16)
        for th in range(2):
            for c in range(DC):
                nc.sync.dma_start(
                    out=xt_sb[:, c, th * 1024:(th + 1) * 1024],
                    in_=xt[c * 128:(c + 1) * 128, th * 1024:(th + 1) * 1024],
                )

        qT = big.tile([64, T], BF16)
        kT = big.tile([64, T], BF16)
        vT = big.tile([64, T], F32R)
        v_sb = big.tile([128, NJT, H + 2], BF16)
        nc.sync.dma_start(out=v_sb[:, :, H:H + 2], in_=vtail[:])
        out_nat = big.tile([128, NJT, H + 2], F32)
        recip = const.tile([128, NJT], F32)

        # PE warmup + ACT table preload during the input-DMA window
        warm = const.tile([128, 512], BF16)
        nc.vector.memset(warm[:], 0.0)
        escr = const.tile([128, 2], F32)
        nc.vector.memset(escr[:], 0.0)
        nc.scalar.activation(
            out=escr[:], in_=escr[:], func=AF.Exp, scale=1.0,
        )
        ps_w = pss.tile([128, 512], F32, tag="s")
        for _ in range(24):
            nc.tensor.matmul(ps_w[:], warm[:, 0:128], warm[:],
                             start=True, stop=True)

        qkv = (qT, kT, vT, v_sb, ident, pss)
        wargs = (xt_sb, wqk_sb, wv_sb, bqk_sb, bv_sb)
        attn = (qT, kT, v_sb, ps, out_ps, ppool)
        drain = (ident, pss, out_nat, recip, out)

        _, vfill0 = _emit_qkv_half(nc, 0, *wargs, *qkv, defer_v=True)
        vfill0[0]()  # v group for jt 0-3: needed at round 0
        vfill0[1]()
        qkc1, vfill1 = _emit_qkv_half(nc, 1, *wargs, *qkv,
                                      defer_v=True, defer_qk=True)
        o0, o1 = _emit_attn_pair(nc, 0, 1, *attn,
                                 fillers=vfill0[2:] + qkc1)
        t0 = _emit_drain_copy(nc, otpool, o0)
        t1 = _emit_drain_copy(nc, otpool, o1)
        fillers = (vfill1
                   + _drain_closures(nc, 0, t0, *drain)
                   + _drain_closures(nc, 1, t1, *drain))
        o2, o3 = _emit_attn_pair(nc, 2, 3, *attn, fillers=fillers)
        t2 = _emit_drain_copy(nc, otpool, o2)
        t3 = _emit_drain_copy(nc, otpool, o3)
        _emit_drain(nc, 2, t2, *drain)
        _emit_drain(nc, 3, t3, *drain)

    nc.compile()
    return nc


def _get_nc():
    if "nc" not in _CACHE:
        _CACHE["nc"] = _build()
    return _CACHE["nc"]


def kernel(x, Wq, bq, Wk, bk, Wv, bv):
    x = np.ascontiguousarray(np.asarray(x, dtype=np.float32))
    Wq = np.asarray(Wq, dtype=np.float32)
    Wk = np.asarray(Wk, dtype=np.float32)
    Wv = np.ascontiguousarray(np.asarray(Wv, dtype=np.float32))
    bq = np.asarray(bq, dtype=np.float32)
    bk = np.asarray(bk, dtype=np.float32)
    bv = np.asarray(bv, dtype=np.float32)

    wqk = np.ascontiguousarray(
        np.concatenate([Wq, Wk], axis=1)).astype(ml_dtypes.bfloat16)
    wv_b = Wv.astype(ml_dtypes.bfloat16)
    x_b = x.astype(ml_dtypes.bfloat16)
    bqk = np.ascontiguousarray(np.concatenate([bq, bk])[:, None])
    bv_ = np.ascontiguousarray(bv[:, None])
    ident66 = np.eye(66, dtype=np.float32)
    vtail = np.zeros((128, NJT, 2), dtype=ml_dtypes.bfloat16)
    vtail[:, :, 0] = 1.0

    in_maps = []
    for b in range(NB):
        in_maps.append({
            "xt": np.ascontiguousarray(x_b[b].T),
            "wqk": wqk,
            "wv": wv_b,
            "bqk": bqk,
            "bv": bv_,
            "ident66": ident66,
            "vtail": vtail,
        })

    nc = _get_nc()
    trace = bool(int(os.environ.get("KTRACE", "0")))
    res = run_bass_kernel_spmd(
        nc, in_maps, core_ids=list(range(NB)), trace=trace,
    )
    if trace:
        _CACHE["exec_time_ns"] = res.exec_time_ns
        _CACHE["results"] = res
    return np.stack([r["out"] for r in res.results])



# revision 4
# speedup vs baseline: 1.0962x; 1.0962x over previous
"""Causal single-head attention on 8 Trainium2 NeuronCores.

Problem: x[8, 2048, 1024] -> out[8, 2048, 64]
  q/k/v = x @ W{q,k,v} + b{q,k,v};  out = softmax(causal(q k^T / 8)) v

Sharding: data-parallel over batch; core b computes batch element b.

Per-core design (T=2048, D=1024, H=64), all matmul operands bf16 with
fp32 PSUM accumulation:
  - host packs ONE bf16 blob [128, 18080] = wqk | wv | ident | vtail | x^T
    plus a [128, 2] f32 bias blob; 8 merged DMA issues (bias, wqk, x
    quarter 0 in two halves, wv+ident+vtail, x quarters 1-3) so the
    first QKV matmul starts as early as possible.
  - QKV per 512-col t-group: lhsT = wqk chunk [128d, 128] -> qT/kT
    [64, T]; V is COL-TILED: two M=64 matmuls at tile_position (0,0)
    and (0,64) compute v^T for the two 256-col halves of a quarter
    concurrently on disjoint PE column groups.
  - v^T tiles PE-transposed in bf16 (cheap) to natural v [128t, 64h]
    + ones/zeros columns -> v_sb [128, 16, 66].
  - attention in jt-PAIR rounds per i-quarter (512 wide): S^T for two
    j-chunks -> one 2-bank PSUM tile [128, 1024]; ONE exp ACTIVATE per
    round (scale=1/8 fused) -> P bf16; causal diagonal blocks masked by
    affine_select; PV accumulates out^T[66, 512] per quarter, whose
    row 64 is the softmax denominator (ones column of v_sb).
  - drain per quarter: out^T cast to bf16, PE-transposed (bf16) to
    natural [128, 4, 66] PSUM, reciprocal of row 64, per-t-tile scale,
    DMA out. Drains and later-quarter QKV work run as fillers inside
    earlier rounds to keep the PE busy during exp waits.
"""

import os
from contextlib import ExitStack

import ml_dtypes
import numpy as np

import concourse.bacc as bacc
import concourse.mybir as mybir
import concourse.tile as tile
from concourse.bass_utils import run_bass_kernel_spmd

F32 = mybir.dt.float32
BF16 = mybir.dt.bfloat16
AF = mybir.ActivationFunctionType
ALU = mybir.AluOpType

T = 2048
D = 1024
H = 64
NB = 8
DC = D // 128       # 8 contraction chunks
NJT = T // 128      # 16 j-chunks (also 16 t-tiles)
QW = 512            # i-quarter width
NQ = T // QW        # 4 quarters
SCALE = 1.0 / 8.0   # 1/sqrt(H)

# blob column offsets (bf16 elements per partition)
OFF_WQK = 0            # [128, 8, 128] -> 1024
OFF_WV = 1024          # [128, 8, 64]  -> 512
OFF_ID = 1536          # [128, 128]    -> 128
OFF_VT = 1664          # [128, 16, 2]  -> 32
OFF_X = 1696           # [128, 8, 2048] -> 16384
BLOB_W = OFF_X + DC * T

_CACHE: dict = {}


def _build():
    nc = bacc.Bacc("TRN2", target_bir_lowering=False, debug=False,
                   num_devices=NB)
    wx = nc.dram_tensor("wx", [128, BLOB_W], BF16, kind="ExternalInput")
    bias2 = nc.dram_tensor("bias2", [128, 2], F32, kind="ExternalInput")
    out = nc.dram_tensor("out", [T, H], F32, kind="ExternalOutput")

    with ExitStack() as ctx:
        tc = ctx.enter_context(tile.TileContext(nc))
        const = ctx.enter_context(tc.tile_pool(name="const", bufs=1))
        big = ctx.enter_context(tc.tile_pool(name="big", bufs=1))
        ppool = ctx.enter_context(tc.tile_pool(name="ppool", bufs=4))
        otpool = ctx.enter_context(tc.tile_pool(name="otpool", bufs=2))
        psw = ctx.enter_context(tc.tile_pool(name="psw", bufs=2, space="PSUM"))
        pss2 = ctx.enter_context(
            tc.tile_pool(name="pss2", bufs=2, space="PSUM"))
        out_ps = ctx.enter_context(
            tc.tile_pool(name="out_ps", bufs=2, space="PSUM"))

        # ---- SBUF tiles ----
        W = const.tile([128, OFF_VT], BF16)        # wqk | wv | ident
        bias = const.tile([128, 2], F32)
        xt_sb = big.tile([128, DC, T], BF16)
        qT = big.tile([64, T], BF16)
        kT = big.tile([64, T], BF16)
        vT = big.tile([64, T], BF16)
        v_sb = big.tile([128, NJT, H + 2], BF16)
        out_sb = big.tile([128, NJT, H], F32)
        recip = big.tile([128, NJT], F32)

        xsrc = wx[:, OFF_X:].rearrange("p (c t) -> p c t", c=DC)

        # ---- input DMAs (order = priority; sync FIFO executes in order)
        nc.sync.dma_start(out=bias[:], in_=bias2[:])
        nc.sync.dma_start(out=W[:, 0:OFF_WV], in_=wx[:, 0:OFF_WV])
        # x quarter 0 in two chunk-halves for earliest QKV start
        nc.sync.dma_start(out=xt_sb[:, 0:4, 0:QW], in_=xsrc[:, 0:4, 0:QW])
        nc.sync.dma_start(out=xt_sb[:, 4:8, 0:QW], in_=xsrc[:, 4:8, 0:QW])
        # wv + ident
        nc.sync.dma_start(out=W[:, OFF_WV:OFF_VT], in_=wx[:, OFF_WV:OFF_VT])
        # vtail -> v_sb ones/zero columns
        nc.sync.dma_start(
            out=v_sb[:, :, H:H + 2],
            in_=wx[:, OFF_VT:OFF_X].rearrange("p (t two) -> p t two", two=2))
        for q in range(1, NQ):
            nc.sync.dma_start(
                out=xt_sb[:, :, q * QW:(q + 1) * QW],
                in_=xsrc[:, :, q * QW:(q + 1) * QW])

        ident = W[:, OFF_ID:OFF_ID + 128]

        # ---- PE warmup + ACT table preload during the input-DMA window
        warm = const.tile([128, 512], BF16)
        nc.vector.memset(warm[:], 0.0)
        escr = const.tile([128, 2], F32)
        nc.vector.memset(escr[:], 0.0)
        nc.scalar.activation(out=escr[:], in_=escr[:], func=AF.Exp, scale=1.0)
        ps_w = pss2.tile([128, 2 * QW], F32, tag="s")
        for _ in range(22):
            nc.tensor.matmul(ps_w[:, 0:QW], warm[:, 0:128], warm[:],
                             start=True, stop=True)

        # ---- QKV emitters ----
        def emit_qk_group(g, c_lo, c_hi):
            """q/k for t in [512g, 512(g+1)), chunks [c_lo, c_hi)."""
            sl = slice(g * QW, (g + 1) * QW)
            ps = emit_qk_group.ps
            if c_lo == 0:
                ps = emit_qk_group.ps = psw.tile(
                    [128, QW], F32, tag="w", name=f"psqk{g}")
            for c in range(c_lo, c_hi):
                nc.tensor.matmul(
                    ps[:], W[:, c * 128:(c + 1) * 128], xt_sb[:, c, sl],
                    start=(c == 0), stop=(c == DC - 1),
                )
            if c_hi == DC:
                nc.vector.tensor_scalar(
                    out=qT[:, sl], in0=ps[0:64, :],
                    scalar1=bias[0:64, 0:1], scalar2=None, op0=ALU.add)
                nc.vector.tensor_scalar(
                    out=kT[:, sl], in0=ps[64:128, :],
                    scalar1=bias[64:128, 0:1], scalar2=None, op0=ALU.add)
        emit_qk_group.ps = None

        def emit_v_quarter(g):
            """v^T for t in [512g, 512(g+1)); col-tiled 2x256."""
            lo = g * QW
            ps = psw.tile([128, QW], F32, tag="w", name=f"psv{g}")
            for c in range(DC):
                wv_c = W[:, OFF_WV + c * H:OFF_WV + (c + 1) * H]
                nc.tensor.matmul(
                    ps[0:64, 0:256], wv_c, xt_sb[:, c, lo:lo + 256],
                    start=(c == 0), stop=(c == DC - 1))
                nc.tensor.matmul(
                    ps[64:128, 0:256], wv_c, xt_sb[:, c, lo + 256:lo + 512],
                    start=(c == 0), stop=(c == DC - 1))
            nc.vector.tensor_scalar(
                out=vT[:, lo:lo + 256], in0=ps[0:64, 0:256],
                scalar1=bias[0:64, 1:2], scalar2=None, op0=ALU.add)
            nc.vector.tensor_scalar(
                out=vT[:, lo + 256:lo + 512], in0=ps[64:128, 0:256],
                scalar1=bias[64:128, 1:2], scalar2=None, op0=ALU.add)

        def emit_vtrans(g):
            """transpose v^T [64,128] tiles -> natural v tiles for quarter g."""
            ps = psw.tile([128, 4, H], BF16, tag="w", name=f"psvt{g}")
            for j2 in range(4):
                jt = 4 * g + j2
                nc.tensor.transpose(
                    ps[:, j2, :], vT[:, jt * 128:(jt + 1) * 128],
                    ident[0:64, 0:64])
            nc.vector.tensor_copy(v_sb[:, 4 * g:4 * g + 4, 0:H], ps[:, :, :])

        # ---- attention round ----
        def emit_round(q, p, ps_o, n_pairs, fillers):
            jt0, jt1 = 2 * p, 2 * p + 1
            off0 = max(0, 128 * jt0 - QW * q)
            off1 = max(0, 128 * jt1 - QW * q)
            ps_s = pss2.tile([128, 2 * QW], F32, tag="s")
            nc.tensor.matmul(
                ps_s[:, off0:QW],
                kT[:, jt0 * 128:(jt0 + 1) * 128],
                qT[:, QW * q + off0:QW * (q + 1)],
                start=True, stop=True)
            nc.tensor.matmul(
                ps_s[:, QW + off1:2 * QW],
                kT[:, jt1 * 128:(jt1 + 1) * 128],
                qT[:, QW * q + off1:QW * (q + 1)],
                start=True, stop=True)
            P = ppool.tile([128, 2 * QW], BF16, tag="P")
            nc.scalar.activation(
                out=P[:, off0:], in_=ps_s[:, off0:], func=AF.Exp, scale=SCALE)
            if jt0 >= 4 * q:
                nc.gpsimd.affine_select(
                    out=P[:, off0:off0 + 128], in_=P[:, off0:off0 + 128],
                    compare_op=ALU.is_ge, fill=0.0,
                    base=0, pattern=[[1, 128]], channel_multiplier=-1)
            if jt1 >= 4 * q:
                nc.gpsimd.affine_select(
                    out=P[:, QW + off1:QW + off1 + 128],
                    in_=P[:, QW + off1:QW + off1 + 128],
                    compare_op=ALU.is_ge, fill=0.0,
                    base=0, pattern=[[1, 128]], channel_multiplier=-1)
            nc.tensor.matmul(
                ps_o[:, off0:QW], v_sb[:, jt0, :], P[:, off0:QW],
                start=(p == 0), stop=False)
            nc.tensor.matmul(
                ps_o[:, off1:QW], v_sb[:, jt1, :], P[:, QW + off1:2 * QW],
                start=False, stop=(p == n_pairs - 1))
            if fillers:
                fillers.pop(0)()

        # ---- drain ----
        def drain_closures(q, ps_o):
            state = {}

            def _copy():
                oT = otpool.tile([66, QW], BF16, tag="oT", name=f"oT{q}")
                state["oT"] = oT
                nc.vector.tensor_copy(oT[:], ps_o[:])

            def _tr(t2):
                def go():
                    if "psn" not in state:
                        state["psn"] = psw.tile(
                            [128, 4, H + 2], BF16, tag="w", name=f"psn{q}")
                    nc.tensor.transpose(
                        state["psn"][:, t2, 0:66],
                        state["oT"][:, t2 * 128:(t2 + 1) * 128],
                        ident[0:66, 0:66])
                return go

            def _fin():
                psn = state["psn"]
                sl = slice(q * 4, (q + 1) * 4)
                nc.vector.reciprocal(recip[:, sl], psn[:, :, H])
                for t2 in range(4):
                    nc.vector.tensor_scalar_mul(
                        out_sb[:, 4 * q + t2, :], psn[:, t2, 0:H],
                        recip[:, 4 * q + t2:4 * q + t2 + 1])
                nc.sync.dma_start(
                    out=out.rearrange(
                        "(qq tt p) h -> qq p tt h", qq=NQ, p=128)[q],
                    in_=out_sb[:, sl, :])

            return [_copy] + [_tr(t) for t in range(4)] + [_fin]

        # ---- emission schedule ----
        # critical path for quarter 0
        emit_qk_group(0, 0, 4)
        emit_qk_group(0, 4, 8)
        emit_v_quarter(0)
        emit_vtrans(0)

        def qk_closures(g):
            return [lambda: emit_qk_group(g, 0, 4),
                    lambda: emit_qk_group(g, 4, 8)]

        drains = {}
        for q in range(NQ):
            n_pairs = 2 * (q + 1)
            ps_o = out_ps.tile([H + 2, QW], F32, tag="out", name=f"pso{q}")
            # fillers: work needed by quarter q+1, plus drain of quarter q-1
            fillers = []
            if q + 1 < NQ:
                g = q + 1
                fillers += qk_closures(g)
                fillers += [lambda g=g: emit_v_quarter(g),
                            lambda g=g: emit_vtrans(g)]
            if q - 1 in drains:
                fillers += drains.pop(q - 1)
            for p in range(n_pairs):
                emit_round(q, p, ps_o, n_pairs, fillers)
            # any unfired fillers must still run before quarter q+1
            for f in fillers:
                f()
            drains[q] = drain_closures(q, ps_o)
        for q, cls in sorted(drains.items()):
            for f in cls:
                f()

    nc.compile()
    return nc


def _get_nc():
    if "nc" not in _CACHE:
        _CACHE["nc"] = _build()
    return _CACHE["nc"]


def kernel(x, Wq, bq, Wk, bk, Wv, bv):
    x = np.ascontiguousarray(np.asarray(x, dtype=np.float32))
    Wq = np.asarray(Wq, dtype=np.float32)
    Wk = np.asarray(Wk, dtype=np.float32)
    Wv = np.ascontiguousarray(np.asarray(Wv, dtype=np.float32))
    bq = np.asarray(bq, dtype=np.float32)
    bk = np.asarray(bk, dtype=np.float32)
    bv = np.asarray(bv, dtype=np.float32)

    bf = ml_dtypes.bfloat16
    # wqk: [1024, 128] -> [128p, 8c, 128m]
    wqk = np.concatenate([Wq, Wk], axis=1).reshape(DC, 128, 128)
    wqk = np.transpose(wqk, (1, 0, 2)).reshape(128, DC * 128)
    # wv: [1024, 64] -> [128p, 8c, 64m]
    wv = Wv.reshape(DC, 128, H)
    wv = np.transpose(wv, (1, 0, 2)).reshape(128, DC * H)
    ident = np.eye(128, dtype=np.float32)
    vtail = np.zeros((128, NJT, 2), dtype=np.float32)
    vtail[:, :, 0] = 1.0
    head = np.concatenate(
        [wqk, wv, ident, vtail.reshape(128, 2 * NJT)], axis=1).astype(bf)

    bias2 = np.zeros((128, 2), dtype=np.float32)
    bias2[:, 0] = np.concatenate([bq, bk])
    bias2[:, 1] = np.concatenate([bv, bv])

    in_maps = []
    for b in range(NB):
        # x[b].T: [1024, 2048] -> [128p, 8c, 2048t]
        xt = np.ascontiguousarray(x[b].T).reshape(DC, 128, T)
        xt = np.transpose(xt, (1, 0, 2)).reshape(128, DC * T).astype(bf)
        blob = np.concatenate([head, xt], axis=1)
        in_maps.append({
            "wx": np.ascontiguousarray(blob),
            "bias2": bias2,
        })

    nc = _get_nc()
    trace = bool(int(os.environ.get("KTRACE", "0")))
    res = run_bass_kernel_spmd(
        nc, in_maps, core_ids=list(range(NB)), trace=trace,
    )
    if trace:
        _CACHE["exec_time_ns"] = res.exec_time_ns
        _CACHE["results"] = res
    return np.stack([r["out"] for r in res.results])


# revision 6
# speedup vs baseline: 1.1152x; 1.0174x over previous
"""Causal single-head attention on 8 Trainium2 NeuronCores.

Problem: x[8, 2048, 1024] -> out[8, 2048, 64]
  q/k/v = x @ W{q,k,v} + b{q,k,v};  out = softmax(causal(q k^T / 8)) v

Sharding: data-parallel over batch; core b computes batch element b.

Per-core design (T=2048, D=1024, H=64), all matmul operands bf16 with
fp32 PSUM accumulation:
  - host packs ONE bf16 blob [128, 18080] = wqk | wv | ident | vtail | x^T
    plus a [128, 2] f32 bias blob; 8 merged DMA issues (bias, wqk, x
    quarter 0 in two halves, wv+ident+vtail, x quarters 1-3) so the
    first QKV matmul starts as early as possible.
  - QKV per 512-col t-group: lhsT = wqk chunk [128d, 128] -> qT/kT
    [64, T]; V is COL-TILED: two M=64 matmuls at tile_position (0,0)
    and (0,64) compute v^T for the two 256-col halves of a quarter
    concurrently on disjoint PE column groups.
  - v^T tiles PE-transposed in bf16 (cheap) to natural v [128t, 64h]
    + ones/zeros columns -> v_sb [128, 16, 66].
  - attention in jt-PAIR rounds per i-quarter (512 wide): S^T for two
    j-chunks -> one 2-bank PSUM tile [128, 1024]; ONE exp ACTIVATE per
    round (scale=1/8 fused) -> P bf16; causal diagonal blocks masked by
    affine_select; PV accumulates out^T[66, 512] per quarter, whose
    row 64 is the softmax denominator (ones column of v_sb).
  - drain per quarter: out^T cast to bf16, PE-transposed (bf16) to
    natural [128, 4, 66] PSUM, reciprocal of row 64, per-t-tile scale,
    DMA out. Drains and later-quarter QKV work run as fillers inside
    earlier rounds to keep the PE busy during exp waits.
"""

import os
from contextlib import ExitStack

import ml_dtypes
import numpy as np

import concourse.bacc as bacc
import concourse.mybir as mybir
import concourse.tile as tile
from concourse.bass_utils import run_bass_kernel_spmd

F32 = mybir.dt.float32
BF16 = mybir.dt.bfloat16
AF = mybir.ActivationFunctionType
ALU = mybir.AluOpType

T = 2048
D = 1024
H = 64
NB = 8
DC = D // 128       # 8 contraction chunks
NJT = T // 128      # 16 j-chunks (also 16 t-tiles)
QW = 512            # i-quarter width
NQ = T // QW        # 4 quarters
SCALE = 1.0 / 8.0   # 1/sqrt(H)

# blob column offsets (bf16 elements per partition)
OFF_WQK = 0            # [128, 8, 128] -> 1024
OFF_WV = 1024          # [128, 8, 64]  -> 512
OFF_ID = 1536          # [128, 128]    -> 128
OFF_VT = 1664          # [128, 16, 2]  -> 32
OFF_X = 1696           # [128, 8, 2048] -> 16384
BLOB_W = OFF_X + DC * T

_CACHE: dict = {}


def _build():
    nc = bacc.Bacc("TRN2", target_bir_lowering=False, debug=False,
                   num_devices=NB)
    wx = nc.dram_tensor("wx", [128, BLOB_W], BF16, kind="ExternalInput")
    bias2 = nc.dram_tensor("bias2", [128, 2], F32, kind="ExternalInput")
    out = nc.dram_tensor("out", [T, H], F32, kind="ExternalOutput")

    with ExitStack() as ctx:
        tc = ctx.enter_context(tile.TileContext(nc))
        const = ctx.enter_context(tc.tile_pool(name="const", bufs=1))
        big = ctx.enter_context(tc.tile_pool(name="big", bufs=1))
        ppool = ctx.enter_context(tc.tile_pool(name="ppool", bufs=4))
        otpool = ctx.enter_context(tc.tile_pool(name="otpool", bufs=2))
        psw = ctx.enter_context(tc.tile_pool(name="psw", bufs=2, space="PSUM"))
        pss2 = ctx.enter_context(
            tc.tile_pool(name="pss2", bufs=2, space="PSUM"))
        out_ps = ctx.enter_context(
            tc.tile_pool(name="out_ps", bufs=2, space="PSUM"))

        # ---- SBUF tiles ----
        W = const.tile([128, OFF_VT], BF16)        # wqk | wv | ident
        bias = const.tile([128, 2], F32)
        xt_sb = big.tile([128, NQ, DC, QW], BF16)
        qT = big.tile([64, T], BF16)
        kT = big.tile([64, T], BF16)
        vT = big.tile([64, T], BF16)
        v_sb = big.tile([128, NJT, H + 2], BF16)
        out_sb = big.tile([128, NJT, H], F32)
        recip = big.tile([128, NJT], F32)

        xsrc = wx[:, OFF_X:].rearrange(
            "p (q c t) -> p q c t", q=NQ, c=DC)

        # ---- input DMAs (order = priority; sync FIFO executes in order)
        nc.sync.dma_start(out=W[:, 0:OFF_WV], in_=wx[:, 0:OFF_WV])
        # x quarter 0 in two chunk-halves for earliest QKV start
        nc.sync.dma_start(out=xt_sb[:, 0, 0:4, :], in_=xsrc[:, 0, 0:4, :])
        nc.sync.dma_start(out=xt_sb[:, 0, 4:8, :], in_=xsrc[:, 0, 4:8, :])
        # wv + ident
        nc.sync.dma_start(out=W[:, OFF_WV:OFF_VT], in_=wx[:, OFF_WV:OFF_VT])
        nc.sync.dma_start(out=bias[:], in_=bias2[:])
        # vtail -> v_sb ones/zero columns
        nc.sync.dma_start(
            out=v_sb[:, :, H:H + 2],
            in_=wx[:, OFF_VT:OFF_X].rearrange("p (t two) -> p t two", two=2))
        for q in range(1, NQ):
            nc.sync.dma_start(out=xt_sb[:, q], in_=xsrc[:, q])

        ident = W[:, OFF_ID:OFF_ID + 128]

        # ---- PE warmup + ACT table preload during the input-DMA window
        warm = const.tile([128, 512], BF16)
        nc.vector.memset(warm[:], 0.0)
        escr = const.tile([128, 2], F32)
        nc.vector.memset(escr[:], 0.0)
        nc.scalar.activation(out=escr[:], in_=escr[:], func=AF.Exp, scale=1.0)
        ps_w = pss2.tile([128, 2 * QW], F32, tag="s")
        for _ in range(5):
            nc.tensor.matmul(ps_w[:, 0:QW], warm[:, 0:128], warm[:],
                             start=True, stop=True)

        # ---- QKV emitters ----
        def emit_qk_group(g, c_lo, c_hi):
            """q/k for t in [512g, 512(g+1)), chunks [c_lo, c_hi)."""
            sl = slice(g * QW, (g + 1) * QW)
            ps = emit_qk_group.ps
            if c_lo == 0:
                ps = emit_qk_group.ps = psw.tile(
                    [128, QW], F32, tag="w", name=f"psqk{g}")
            for c in range(c_lo, c_hi):
                nc.tensor.matmul(
                    ps[:], W[:, c * 128:(c + 1) * 128], xt_sb[:, g, c, :],
                    start=(c == 0), stop=(c == DC - 1),
                )
            if c_hi == DC:
                nc.vector.tensor_scalar(
                    out=qT[:, sl], in0=ps[0:64, :],
                    scalar1=bias[0:64, 0:1], scalar2=None, op0=ALU.add)
                nc.vector.tensor_scalar(
                    out=kT[:, sl], in0=ps[64:128, :],
                    scalar1=bias[64:128, 0:1], scalar2=None, op0=ALU.add)
        emit_qk_group.ps = None

        def emit_v_quarter(g):
            """v^T for t in [512g, 512(g+1)); col-tiled 2x256."""
            lo = g * QW
            ps = psw.tile([128, QW], F32, tag="w", name=f"psv{g}")
            for c in range(DC):
                wv_c = W[:, OFF_WV + c * H:OFF_WV + (c + 1) * H]
                nc.tensor.matmul(
                    ps[0:64, 0:256], wv_c, xt_sb[:, g, c, 0:256],
                    start=(c == 0), stop=(c == DC - 1))
                nc.tensor.matmul(
                    ps[64:128, 0:256], wv_c, xt_sb[:, g, c, 256:512],
                    start=(c == 0), stop=(c == DC - 1))
            nc.vector.tensor_scalar(
                out=vT[:, lo:lo + 256], in0=ps[0:64, 0:256],
                scalar1=bias[0:64, 1:2], scalar2=None, op0=ALU.add)
            nc.vector.tensor_scalar(
                out=vT[:, lo + 256:lo + 512], in0=ps[64:128, 0:256],
                scalar1=bias[64:128, 1:2], scalar2=None, op0=ALU.add)

        def emit_vtrans(g):
            """transpose v^T [64,128] tiles -> natural v tiles for quarter g."""
            ps = psw.tile([128, 4, H], BF16, tag="w", name=f"psvt{g}")
            for j2 in range(4):
                jt = 4 * g + j2
                nc.tensor.transpose(
                    ps[:, j2, :], vT[:, jt * 128:(jt + 1) * 128],
                    ident[0:64, 0:64])
            nc.vector.tensor_copy(v_sb[:, 4 * g:4 * g + 4, 0:H], ps[:, :, :])

        # ---- attention round ----
        def emit_round(q, p, ps_o, n_pairs, fillers):
            """fillers: list of (cost_us, closure); pops ~one round's worth."""
            jt0, jt1 = 2 * p, 2 * p + 1
            off0 = max(0, 128 * jt0 - QW * q)
            off1 = max(0, 128 * jt1 - QW * q)
            ps_s = pss2.tile([128, 2 * QW], F32, tag="s")
            nc.tensor.matmul(
                ps_s[:, off0:QW],
                kT[:, jt0 * 128:(jt0 + 1) * 128],
                qT[:, QW * q + off0:QW * (q + 1)],
                start=True, stop=True)
            nc.tensor.matmul(
                ps_s[:, QW + off1:2 * QW],
                kT[:, jt1 * 128:(jt1 + 1) * 128],
                qT[:, QW * q + off1:QW * (q + 1)],
                start=True, stop=True)
            P = ppool.tile([128, 2 * QW], BF16, tag="P")
            nc.scalar.activation(
                out=P[:, off0:], in_=ps_s[:, off0:], func=AF.Exp, scale=SCALE)
            if jt0 >= 4 * q:
                nc.gpsimd.affine_select(
                    out=P[:, off0:off0 + 128], in_=P[:, off0:off0 + 128],
                    compare_op=ALU.is_ge, fill=0.0,
                    base=0, pattern=[[1, 128]], channel_multiplier=-1)
            if jt1 >= 4 * q:
                nc.gpsimd.affine_select(
                    out=P[:, QW + off1:QW + off1 + 128],
                    in_=P[:, QW + off1:QW + off1 + 128],
                    compare_op=ALU.is_ge, fill=0.0,
                    base=0, pattern=[[1, 128]], channel_multiplier=-1)
            nc.tensor.matmul(
                ps_o[:, off0:QW], v_sb[:, jt0, :], P[:, off0:QW],
                start=(p == 0), stop=False)
            nc.tensor.matmul(
                ps_o[:, off1:QW], v_sb[:, jt1, :], P[:, QW + off1:2 * QW],
                start=False, stop=(p == n_pairs - 1))
            budget = 0.95
            while fillers and budget > 0:
                w, f = fillers.pop(0)
                f()
                budget -= w

        # ---- drain ----
        def drain_closures(q, ps_o):
            state = {}

            def _copy(h):
                def go():
                    if "oT" not in state:
                        state["oT"] = otpool.tile(
                            [66, QW], BF16, tag="oT", name=f"oT{q}")
                    nc.vector.tensor_copy(
                        state["oT"][:, h * 256:(h + 1) * 256],
                        ps_o[:, h * 256:(h + 1) * 256])
                return go

            def _tr(t2):
                def go():
                    if "psn" not in state:
                        state["psn"] = psw.tile(
                            [128, 4, H + 2], BF16, tag="w", name=f"psn{q}")
                    nc.tensor.transpose(
                        state["psn"][:, t2, 0:66],
                        state["oT"][:, t2 * 128:(t2 + 1) * 128],
                        ident[0:66, 0:66])
                return go

            def _fin():
                psn = state["psn"]
                sl = slice(q * 4, (q + 1) * 4)
                nc.vector.reciprocal(recip[:, sl], psn[:, :, H])
                for t2 in range(4):
                    nc.vector.tensor_scalar_mul(
                        out_sb[:, 4 * q + t2, :], psn[:, t2, 0:H],
                        recip[:, 4 * q + t2:4 * q + t2 + 1])
                nc.sync.dma_start(
                    out=out.rearrange(
                        "(qq tt p) h -> qq p tt h", qq=NQ, p=128)[q],
                    in_=out_sb[:, sl, :])

            return [(0.25, _copy(0)), (0.2, _tr(0)), (0.2, _tr(1)),
                    (0.25, _copy(1)), (0.2, _tr(2)), (0.2, _tr(3)),
                    (0.5, _fin)]

        # ---- emission schedule ----
        # critical path for quarter 0
        emit_qk_group(0, 0, 4)
        emit_qk_group(0, 4, 8)
        emit_v_quarter(0)
        emit_vtrans(0)

        def prep_closures(g):
            return [(0.95, lambda: emit_qk_group(g, 0, 4)),
                    (0.95, lambda: emit_qk_group(g, 4, 8)),
                    (1.1, lambda: emit_v_quarter(g)),
                    (0.9, lambda: emit_vtrans(g))]

        fillers = []
        for q in range(NQ):
            n_pairs = 2 * (q + 1)
            ps_o = out_ps.tile([H + 2, QW], F32, tag="out", name=f"pso{q}")
            if q + 1 < NQ:
                fillers += prep_closures(q + 1)
            for p in range(n_pairs):
                emit_round(q, p, ps_o, n_pairs, fillers)
            # quarter q+1 prep must be complete before its rounds start
            for w, f in fillers:
                f()
            fillers = drain_closures(q, ps_o)
        for w, f in fillers:
            f()

    nc.compile()
    return nc


def _get_nc():
    if "nc" not in _CACHE:
        _CACHE["nc"] = _build()
    return _CACHE["nc"]


def kernel(x, Wq, bq, Wk, bk, Wv, bv):
    x = np.ascontiguousarray(np.asarray(x, dtype=np.float32))
    Wq = np.asarray(Wq, dtype=np.float32)
    Wk = np.asarray(Wk, dtype=np.float32)
    Wv = np.ascontiguousarray(np.asarray(Wv, dtype=np.float32))
    bq = np.asarray(bq, dtype=np.float32)
    bk = np.asarray(bk, dtype=np.float32)
    bv = np.asarray(bv, dtype=np.float32)

    bf = ml_dtypes.bfloat16
    # wqk: [1024, 128] -> [128p, 8c, 128m]
    wqk = np.concatenate([Wq, Wk], axis=1).reshape(DC, 128, 128)
    wqk = np.transpose(wqk, (1, 0, 2)).reshape(128, DC * 128)
    # wv: [1024, 64] -> [128p, 8c, 64m]
    wv = Wv.reshape(DC, 128, H)
    wv = np.transpose(wv, (1, 0, 2)).reshape(128, DC * H)
    ident = np.eye(128, dtype=np.float32)
    vtail = np.zeros((128, NJT, 2), dtype=np.float32)
    vtail[:, :, 0] = 1.0
    head = np.concatenate(
        [wqk, wv, ident, vtail.reshape(128, 2 * NJT)], axis=1).astype(bf)

    bias2 = np.zeros((128, 2), dtype=np.float32)
    bias2[:, 0] = np.concatenate([bq, bk])
    bias2[:, 1] = np.concatenate([bv, bv])

    in_maps = []
    for b in range(NB):
        # x[b].T: [1024, 2048] -> [128p, 4q, 8c, 512t]
        xt = np.ascontiguousarray(x[b].T).reshape(DC, 128, NQ, QW)
        xt = np.transpose(xt, (1, 2, 0, 3)).reshape(128, DC * T).astype(bf)
        blob = np.concatenate([head, xt], axis=1)
        in_maps.append({
            "wx": np.ascontiguousarray(blob),
            "bias2": bias2,
        })

    nc = _get_nc()
    trace = bool(int(os.environ.get("KTRACE", "0")))
    res = run_bass_kernel_spmd(
        nc, in_maps, core_ids=list(range(NB)), trace=trace,
    )
    if trace:
        _CACHE["exec_time_ns"] = res.exec_time_ns
        _CACHE["results"] = res
    return np.stack([r["out"] for r in res.results])
